# revision 17
# baseline (speedup 1.0000x reference)
"""Trainium2 Bass kernel for nn_Channel_Transposed_Attention (B8 C384 H64 W64).

Data-parallel over batch: 8 batch elements -> 8 NeuronCores (SPMD, per-core
x slice). Per core everything lives in (C, N) channel-major layout (N=H*W).
Channel convs/projections use 128-channel tiles (3 per C=384); attention
head structure (8 heads x 48) uses 48/96-partition tiles. q,k are produced
in (N, C) token-major layout; per head the [q_h|k_h] Gram matrix gives both
the attention logits and the l2-norm diagonals in one accumulated matmul
chain. The scaled logits A*diag(rk) are computed on the PE via a per-head
transpose + diagonal matmul (rq folds into the exp scale), avoiding any
partition broadcast. The attention output at = attn @ v runs in 128-channel
tiles via 7 sparse block matrices E[vt,g] assembled from transposed softmax
blocks. Depthwise convs run as PE tap-accumulation with per-channel diagonal
weight matrices (bf16, built on-device) over zero-padded row-strided buffers
(only borders are zeroed). Weights ship as two packed blobs; x streams per
512-token chunk on the GpSimd DMA queue.
"""
import os
import numpy as np
from contextlib import ExitStack

import concourse.bass as bass
import concourse.bacc as bacc
import concourse.tile as tile
from concourse import mybir
from concourse.bass_utils import run_bass_kernel_spmd
from concourse._compat import with_exitstack

import ml_dtypes
BF16 = ml_dtypes.bfloat16

F32 = mybir.dt.float32
BF = mybir.dt.bfloat16
AF = mybir.ActivationFunctionType
OP = mybir.AluOpType
AX = mybir.AxisListType

H = W = 64
N = H * W               # 4096
HP = W + 2              # 66   pad-1 row stride
NP1 = (H + 2) * HP      # 4356
BP = W + 18             # 82   pad-9 row stride (ci2b)
C = 384
C6, C2, C4 = 64, 192, 96
HEADS, HD = 8, 48
NCORES = 8
CH = 512
NCH = N // CH           # 8
QKW = 2 * C             # 768

# softmax pieces: (head, v-tile, col-offset in en_vt, k_lo, k_hi)
PIECES = [(0, 0, 0, 0, 48), (1, 0, 48, 0, 48),
          (2, 0, 96, 0, 32), (2, 1, 0, 32, 48),
          (3, 1, 16, 0, 48), (4, 1, 64, 0, 48),
          (5, 1, 112, 0, 16), (5, 2, 0, 16, 48),
          (6, 2, 32, 0, 48), (7, 2, 80, 0, 48)]
# E-matrix assembly: (vt, g, prow_lo, prow_hi, col_off, c_lo, c_hi)
ECOPY = [(0, 0, 0, 48, 0, 0, 48),        # h0
         (0, 0, 48, 96, 48, 0, 48),      # h1
         (0, 0, 96, 128, 96, 0, 32),     # h2
         (0, 1, 96, 128, 0, 32, 48),
         (1, 0, 0, 16, 96, 0, 32),
         (1, 1, 0, 16, 0, 32, 48),
         (1, 1, 16, 64, 16, 0, 48),      # h3
         (1, 1, 64, 112, 64, 0, 48),     # h4
         (1, 1, 112, 128, 112, 0, 16),   # h5
         (1, 2, 112, 128, 0, 16, 48),
         (2, 1, 0, 32, 112, 0, 16),
         (2, 2, 0, 32, 0, 16, 48),
         (2, 2, 32, 80, 32, 0, 48),      # h6
         (2, 2, 80, 128, 80, 0, 48)]     # h7
EPAIRS = {0: [0, 1], 1: [0, 1, 2], 2: [1, 2]}

_last_results = None


def _win(t, off, dims, p=None):
    """Strided free-dim window of a 2D tile AP at free element offset."""
    base = t[:, off:off + 1] if p is None else t[p[0]:p[1], off:off + 1]
    return bass.AP(tensor=base.tensor, offset=base.offset,
                   ap=[list(base.ap[0])] + [list(dd) for dd in dims])


# ---- packed f32 blob column layout -----------------------------------------
FB = {}
_c = 0
def _fcol(name, n):
    global _c
    FB[name] = _c
    _c += n
_fcol("dw1_b", 3)      # [128] x3
_fcol("dw2_b", 3)
_fcol("cp_in_b", 1)    # [64]
_fcol("ci1_b", 1)
_fcol("ci2a_b2", 1)    # [128]
_fcol("ci2b_b2", 1)
_fcol("ci2c_b", 1)
_fcol("cp_out_b", 3)   # [128] x3
_fcol("sp_in_b", 2)    # [96] x2
_fcol("sp_dw_b", 2)
_fcol("sp_out_b", 3)   # [128] x3
_fcol("proj_b", 3)
_fcol("temp", 8)       # broadcast [96, 8]
_fcol("ci1T", 64)      # [64, 64]
_fcol("dw2c", 27)      # [128, 9] x3 m-major
_fcol("spdwc", 18)     # [96, 9] x2
_fcol("ciac", 9)       # [128, 9]
_fcol("cibc", 49)      # [128, 49]
FCOLS = _c

# ---- packed bf16 blob column layout ----------------------------------------
BB = {}
_c = 0
def _bcol(name, n):
    global _c
    BB[name] = _c
    _c += n
_bcol("dw1T", 3 * C)    # [128, 384] x3 k-tiles
_bcol("projT", 3 * C)   # [128, 384] x3
_bcol("cpinT", 3 * C6)  # [128, 64] x3
_bcol("ci2cT", 2 * C6)  # [128, 64] x2 (halves on partition ranges)
_bcol("cpoutT", C)      # [64, 384]
_bcol("spinT", 3 * C2)  # [128, 192] x3
_bcol("spoutT", C)      # [96, 384]
BCOLS = _c


def build_host_inputs(inputs):
    g = {}
    qkv_w = np.asarray(inputs["qkv_w"], np.float32)
    wt = qkv_w.T                                     # [384, 1152] qk | v
    g["wt"] = np.ascontiguousarray(wt.reshape(3, 128, 3 * C)).astype(BF16)

    bb = np.zeros((128, BCOLS), np.float32)
    def put3(nm, mat, w):
        for k in range(3):
            bb[:, BB[nm] + w * k:BB[nm] + w * (k + 1)] = \
                mat[128 * k:128 * (k + 1), :]
    put3("dw1T", np.asarray(inputs["dw1_w"], np.float32).reshape(C, C).T, C)
    put3("projT", np.asarray(inputs["proj_w"], np.float32).T, C)
    put3("cpinT", np.asarray(inputs["cp_in_w"], np.float32).reshape(C6, C).T,
         C6)
    put3("spinT", np.asarray(inputs["sp_in_w"], np.float32).reshape(C2, C).T,
         C2)
    ci2cT = np.asarray(inputs["ci2c_w"], np.float32).reshape(C6, C6).T
    bb[0:64, BB["ci2cT"]:BB["ci2cT"] + C6] = ci2cT
    bb[64:128, BB["ci2cT"] + C6:BB["ci2cT"] + 2 * C6] = ci2cT
    bb[0:64, BB["cpoutT"]:BB["cpoutT"] + C] = \
        np.asarray(inputs["cp_out_w"], np.float32).reshape(C, C6).T
    bb[0:96, BB["spoutT"]:BB["spoutT"] + C] = \
        np.asarray(inputs["sp_out_w"], np.float32).reshape(C, C4).T
    g["blob16"] = np.ascontiguousarray(bb).astype(BF16)

    fb = np.zeros((128, FCOLS), np.float32)
    def put(nm, vec, p=None):
        vec = np.asarray(vec, np.float32).reshape(-1)
        fb[0:len(vec), FB[nm] + (p or 0)] = vec
    for m in range(3):
        put("dw1_b", inputs["dw1_b"][128 * m:128 * (m + 1)], m)
        put("dw2_b", inputs["dw2_b"][128 * m:128 * (m + 1)], m)
        put("cp_out_b", inputs["cp_out_b"][128 * m:128 * (m + 1)], m)
        put("sp_out_b", inputs["sp_out_b"][128 * m:128 * (m + 1)], m)
        put("proj_b", inputs["proj_b"][128 * m:128 * (m + 1)], m)
    for m in range(2):
        put("sp_in_b", inputs["sp_in_b"][96 * m:96 * (m + 1)], m)
        put("sp_dw_b", inputs["sp_dw_b"][96 * m:96 * (m + 1)], m)
    put("cp_in_b", inputs["cp_in_b"])
    put("ci1_b", inputs["ci1_b"])
    put("ci2a_b2", np.tile(np.asarray(inputs["ci2a_b"], np.float32), 2))
    put("ci2b_b2", np.tile(np.asarray(inputs["ci2b_b"], np.float32), 2))
    put("ci2c_b", inputs["ci2c_b"])
    temp = np.asarray(inputs["temperature"], np.float32).reshape(1, HEADS)
    fb[0:96, FB["temp"]:FB["temp"] + 8] = np.broadcast_to(temp, (96, 8))
    fb[0:64, FB["ci1T"]:FB["ci1T"] + 64] = \
        np.asarray(inputs["ci1_w"], np.float32).reshape(C6, C6).T
    dw2 = np.asarray(inputs["dw2_w"], np.float32).reshape(C, 9)
    for m in range(3):
        fb[:, FB["dw2c"] + 9 * m:FB["dw2c"] + 9 * (m + 1)] = \
            dw2[128 * m:128 * (m + 1)]
    spdw = np.asarray(inputs["sp_dw_w"], np.float32).reshape(C2, 9)
    for m in range(2):
        fb[0:96, FB["spdwc"] + 9 * m:FB["spdwc"] + 9 * (m + 1)] = \
            spdw[96 * m:96 * (m + 1)]
    cia = np.asarray(inputs["ci2a_w"], np.float32).reshape(C6, 9)
    fb[:, FB["ciac"]:FB["ciac"] + 9] = np.vstack([cia, cia])
    cib = np.asarray(inputs["ci2b_w"], np.float32).reshape(C6, 49)
    fb[:, FB["cibc"]:FB["cibc"] + 49] = np.vstack([cib, cib])
    g["blob32"] = np.ascontiguousarray(fb)
    return g


@with_exitstack
def emit(ctx: ExitStack, tc, d):
    nc = tc.nc
    sync = nc.sync
    gp = nc.gpsimd

    # ---- LEFT: one persistent pool, all tiles created up front -------------
    wp = ctx.enter_context(tc.tile_pool(name="wp", bufs=1, side="left"))

    # RIGHT stack bottom: v (phase1 -> at)
    vstack = ExitStack()
    vp = vstack.enter_context(tc.tile_pool(name="vp", bufs=1, side="right"))
    v_sb = [vp.tile([128, N], BF, name=f"v{m}") for m in range(3)]

    # RIGHT: phase-1 transients (wt + x ring + qkt ring)
    p1stack = ExitStack()
    xw = p1stack.enter_context(tc.tile_pool(name="xw", bufs=1, side="right"))
    xring = p1stack.enter_context(tc.tile_pool(name="xring", bufs=3,
                                               side="right"))
    qkring = p1stack.enter_context(tc.tile_pool(name="qkring", bufs=3,
                                                side="right"))
    wt_sb = [xw.tile([128, 3 * C], BF, name=f"wt{k}") for k in range(3)]
    for k in range(3):
        sync.dma_start(out=wt_sb[k], in_=d["wt"][k])

    blob16 = wp.tile([128, BCOLS], BF, name="blob16")
    blob32 = wp.tile([128, FCOLS], F32, name="blob32")

    def bview(nm, i, w, p=128):
        return blob16[0:p, BB[nm] + w * i:BB[nm] + w * (i + 1)]

    def fcol(nm, i=0, p=96):
        return blob32[0:p, FB[nm] + i:FB[nm] + i + 1]

    dw1T = [bview("dw1T", k, C) for k in range(3)]
    projT = [bview("projT", k, C) for k in range(3)]
    cpinT = [bview("cpinT", k, C6) for k in range(3)]
    ci2cT = [bview("ci2cT", h, C6) for h in range(2)]
    cpoutT = bview("cpoutT", 0, C, 64)
    spinT = [bview("spinT", k, C2) for k in range(3)]
    spoutT = bview("spoutT", 0, C, 96)
    ci1T = blob32[0:64, FB["ci1T"]:FB["ci1T"] + 64]
    tempb = blob32[0:96, FB["temp"]:FB["temp"] + 8]
    dw1_b = [fcol("dw1_b", m, p=128) for m in range(3)]
    dw2_b = [fcol("dw2_b", m, p=128) for m in range(3)]
    cp_in_b = fcol("cp_in_b", p=64)
    ci1_b = fcol("ci1_b", p=64)
    ci2a_b2 = fcol("ci2a_b2", p=128)
    ci2b_b2 = fcol("ci2b_b2", p=128)
    ci2c_b = fcol("ci2c_b", p=64)
    cp_out_b = [fcol("cp_out_b", m, p=128) for m in range(3)]
    sp_in_b = [fcol("sp_in_b", m) for m in range(2)]
    sp_dw_b = [fcol("sp_dw_b", m) for m in range(2)]
    sp_out_b = [fcol("sp_out_b", m, p=128) for m in range(3)]
    proj_b = [fcol("proj_b", m, p=128) for m in range(3)]

    # LEFT persistents
    i128 = wp.tile([128, 128], F32, name="i128")
    i48b = wp.tile([48, 48], BF, name="i48b")
    ssq = wp.tile([96, HEADS], F32, name="ssq")
    gscr = wp.tile([96, 96], BF, name="gscr")
    rn = wp.tile([96, HEADS], F32, name="rn")
    rnT = wp.tile([HEADS, 96], F32, name="rnT")
    rqs = wp.tile([48, HEADS], F32, name="rqs")
    ssc = wp.tile([48, 4], F32, name="ssc")
    ssum8 = wp.tile([48, HEADS], F32, name="ssum8")
    rs = wp.tile([48, HEADS], F32, name="rs")
    a_sb = [wp.tile([48, 48], F32, name=f"a{h}") for h in range(HEADS)]
    aT_sb = [wp.tile([48, 48], F32, name=f"aT{h}") for h in range(HEADS)]
    dgrk = [wp.tile([48, 48], F32, name=f"dgrk{h}") for h in range(HEADS)]
    rkcol = wp.tile([48, HEADS], F32, name="rkcol")
    en_vt = [wp.tile([48, 128], BF, name=f"envt{v}") for v in range(3)]
    E = {}
    for (vt, gq) in [(0, 0), (0, 1), (1, 0), (1, 1), (1, 2), (2, 1), (2, 2)]:
        E[(vt, gq)] = wp.tile([128, 128], BF, name=f"E{vt}{gq}")
    cm_sig = wp.tile([128, 3], F32, name="cm_sig")
    tsum = wp.tile([C6, NCH], F32, name="tsum")
    tm = wp.tile([C6, 1], F32, name="tm")
    ci1v = wp.tile([C6, 1], F32, name="ci1v")
    pm = wp.tile([C6, NCH], F32, name="pm")
    pmean32 = wp.tile([C6, 1], F32, name="pmean32")
    pmean = wp.tile([C6, 1], BF, name="pmean")
    at_sb = [wp.tile([128, N], BF, name=f"at{m}") for m in range(3)]
    convx = [wp.tile([128, N], BF, name=f"cx{m}") for m in range(3)]
    dd = [wp.tile([96, N], BF, name=f"dd{m}") for m in range(2)]
    sg_ring = [wp.tile([128, CH], BF, name=f"sg{i}") for i in range(3)]
    cbd_ring = [wp.tile([128, CH], BF, name=f"cbd{i}") for i in range(2)]
    oring = [wp.tile([128, CH], F32, name=f"oring{i}") for i in range(4)]
    dacc = [wp.tile([128, CH], BF, name=f"dacc{i}") for i in range(3)]

    # startup on-device constants
    gp.memset(i128, 1.0)
    gp.affine_select(out=i128, in_=i128, pattern=[[-1, 128]], base=0,
                     channel_multiplier=1, compare_op=OP.is_equal, fill=0.0)
    i96 = i128[0:96, 0:96]
    gp.tensor_copy(out=i48b, in_=i128[0:48, 0:48])
    for e in E.values():
        gp.memset(e, 0.0)

    _dgi = [0]

    def mkdiag(pool, p, col):
        _dgi[0] += 1
        t = pool.tile([p, p], BF, name=f"dg{_dgi[0]}")
        if _dgi[0] % 2 == 0:
            nc.vector.tensor_scalar(out=t, in0=i128[0:p, 0:p], scalar1=col,
                                    scalar2=None, op0=OP.mult)
        else:
            nc.scalar.activation(out=t, in_=i128[0:p, 0:p], func=AF.Copy,
                                 scale=col)
        return t

    def pad_borders(t, p, nrow, stride, pw):
        """Zero only the pad borders of an image buffer."""
        eng = nc.vector
        eng.memset(_win(t, 0, [[1, pw * stride]], p=(0, p)), 0.0)
        eng.memset(_win(t, (nrow - pw) * stride, [[1, pw * stride]],
                        p=(0, p)), 0.0)
        eng.memset(_win(t, pw * stride, [[stride, nrow - 2 * pw],
                                         [stride - pw, 2], [1, pw]],
                        p=(0, p)), 0.0)


    _dri = [0]

    def dw_taps(ps, out, taps, diag, wcol_f, src_f, bias, pe_taps):
        """Depthwise conv: first pe_taps taps on PE (PSUM), rest chained on
        Vector; final out = (ps + bias) + acc."""
        for ti in range(pe_taps):
            nc.tensor.matmul(ps, lhsT=diag[taps[ti]], rhs=src_f(taps[ti]),
                             start=(ti == 0), stop=(ti == pe_taps - 1))
        _dri[0] += 1
        acc = dacc[_dri[0] % 3]
        p = ps.partition_size()
        for i, t_ in enumerate(taps[pe_taps:]):
            if i == 0:
                nc.vector.tensor_scalar(out=acc[0:p, :], in0=src_f(t_),
                                        scalar1=wcol_f(t_), scalar2=None,
                                        op0=OP.mult)
            else:
                nc.vector.scalar_tensor_tensor(
                    out=acc[0:p, :], in0=src_f(t_), scalar=wcol_f(t_),
                    in1=acc[0:p, :], op0=OP.mult, op1=OP.add)
        nc.vector.scalar_tensor_tensor(out=out, in0=ps, scalar=bias,
                                       in1=acc[0:p, :], op0=OP.add,
                                       op1=OP.add)

    # evacuation helper: alternate Vector / Scalar engines
    _evi = [0]

    def evac(out, ps, bias=None, accum=None):
        _evi[0] += 1
        if _evi[0] % 3 == 0:
            if bias is None:
                nc.vector.tensor_copy(out=out, in_=ps)
            elif accum is None:
                nc.vector.tensor_scalar(out=out, in0=ps, scalar1=bias,
                                        scalar2=None, op0=OP.add)
            else:
                nc.vector.tensor_scalar(out=out, in0=ps, scalar1=bias,
                                        scalar2=0.0, op0=OP.add, op1=OP.add,
                                        accum_out=accum)
        else:
            if bias is None:
                nc.scalar.copy(out=out, in_=ps)
            else:
                nc.scalar.activation(out=out, in_=ps, func=AF.Identity,
                                     bias=bias, scale=1.0, accum_out=accum)

    # ==== phase 1: qkv + head Grams ========================================
    with tc.tile_pool(name="psQK", bufs=2, space="PSUM") as psQK, \
         tc.tile_pool(name="psV", bufs=2, space="PSUM") as psV, \
         tc.tile_pool(name="psG", bufs=1, space="PSUM") as psG:
        gps = psG.tile([96, HEADS * 96], F32, name="gps")
        for cchunk in range(NCH):
            xc = [xring.tile([128, CH], BF, name=f"xc{k}", tag=f"xc{k}")
                  for k in range(3)]
            for k in range(3):
                nc.scalar.dma_start(
                    out=xc[k], in_=d["x"][128 * k:128 * (k + 1),
                                          CH * cchunk:CH * (cchunk + 1)])
            if cchunk == 1:
                sync.dma_start(out=blob16, in_=d["blob16"][:])
                sync.dma_start(out=blob32, in_=d["blob32"][:])
            for j in range(4):
                i = 4 * cchunk + j
                ps = psQK.tile([128, QKW], F32, name="qkps", tag="qkps")
                for o0, ow in ((0, 512), (512, 256)):
                    for k in range(3):
                        nc.tensor.matmul(
                            ps[:, o0:o0 + ow],
                            lhsT=xc[k][:, 128 * j:128 * (j + 1)],
                            rhs=wt_sb[k][:, o0:o0 + ow],
                            start=(k == 0), stop=(k == 2))
                # store head-interleaved: [h0: q48|k48][h1: q48|k48]...
                qkt = qkring.tile([128, QKW], BF, name="qkt", tag="qkt")
                dst = qkt.rearrange("p (h two f) -> p two h f",
                                    two=2, h=HEADS, f=HD)
                srcv = ps.rearrange("p (two h f) -> p two h f",
                                    two=2, h=HEADS, f=HD)
                if i % 2 == 0:
                    nc.scalar.copy(out=dst, in_=srcv)
                else:
                    nc.vector.tensor_copy(out=dst, in_=srcv)
                for h in range(HEADS):
                    lap = qkt[:, 96 * h:96 * (h + 1)]
                    nc.tensor.matmul(
                        gps[:, 96 * h:96 * (h + 1)], lhsT=lap, rhs=lap,
                        start=(i == 0), stop=(i == 4 * NCH - 1),
                        skip_group_check=True)
            for m in range(3):
                ps = psV.tile([128, CH], F32, name="vps", tag="vps")
                for k in range(3):
                    nc.tensor.matmul(
                        ps,
                        lhsT=wt_sb[k][:, QKW + 128 * m:QKW + 128 * (m + 1)],
                        rhs=xc[k],
                        start=(k == 0), stop=(k == 2))
                nc.scalar.copy(
                    out=v_sb[m][:, CH * cchunk:CH * (cchunk + 1)], in_=ps)

        # per-head: diag -> ssq column; copy A block to SBUF
        for h in range(HEADS):
            nc.vector.scalar_tensor_tensor(
                out=gscr, in0=gps[:, 96 * h:96 * (h + 1)], scalar=1.0,
                in1=i96, op0=OP.mult, op1=OP.mult,
                accum_out=ssq[:, h:h + 1])
            nc.vector.tensor_copy(
                out=a_sb[h], in_=gps[0:48, 96 * h + 48:96 * h + 96])
    p1stack.close()   # free wt + x ring + qkt ring

    # softmax prep on Vector/Scalar (runs under dw1 PE work)
    nc.scalar.activation(out=rn, in_=ssq, func=AF.Sqrt)
    nc.vector.reciprocal(out=rn, in_=rn)
    nc.vector.tensor_mul(rqs, rn[0:48, :], tempb[0:48, :])

    # RIGHT: conv1p pads; dw2 diags live in wp
    cvstack = ExitStack()
    c1pool = cvstack.enter_context(tc.tile_pool(name="c1p", bufs=1,
                                                side="right"))
    conv1p = [c1pool.tile([128, NP1], BF, name=f"c1p{m}") for m in range(3)]
    for m in range(3):
        pad_borders(conv1p[m], 128, H + 2, HP, 1)
    dw2_diag = [[mkdiag(wp, 128, fcol("dw2c", 9 * m + t_, p=128))
                 for t_ in range(9)] for m in range(3)]

    smstack = ExitStack()
    psB = smstack.enter_context(tc.tile_pool(name="psB", bufs=1, space="PSUM"))
    psT2 = smstack.enter_context(tc.tile_pool(name="psT2", bufs=3,
                                              space="PSUM"))
    askp = psB.tile([48, 48 * HEADS], F32, name="askp")

    # ==== dw1 (PE) =========================================================
    with tc.tile_pool(name="psC", bufs=4, space="PSUM") as psC:
        for m in range(3):
            for cchunk in range(NCH):
                ps = psC.tile([128, CH], F32, name="cps", tag="cps")
                for k in range(3):
                    nc.tensor.matmul(
                        ps, lhsT=dw1T[k][:, 128 * m:128 * (m + 1)],
                        rhs=v_sb[k][:, CH * cchunk:CH * (cchunk + 1)],
                        start=(k == 0), stop=(k == 2))
                dst = _win(conv1p[m], (8 * cchunk + 1) * HP + 1,
                           [[HP, 8], [1, W]])
                evac(dst, ps, dw1_b[m])

        # softmax-prep transposes + scaled logits (PE, after dw1)
        psr = psT2.tile([HEADS, 96], F32, name="psr", tag="t")
        nc.tensor.transpose(psr, rn, i96)
        nc.vector.tensor_copy(out=rnT, in_=psr)
        pkc = psT2.tile([48, HEADS], F32, name="pkc", tag="t")
        nc.tensor.transpose(pkc, rnT[0:8, 48:96], i128[0:8, 0:8])
        nc.vector.tensor_copy(out=rkcol, in_=pkc)
        for h in range(HEADS):
            nc.vector.tensor_scalar(out=dgrk[h], in0=i128[0:48, 0:48],
                                    scalar1=rkcol[:, h:h + 1], scalar2=None,
                                    op0=OP.mult)
        for h in range(HEADS):
            aTps = psT2.tile([48, 48], F32, name="aTps", tag="t")
            nc.tensor.transpose(aTps, a_sb[h], i128[0:48, 0:48])
            nc.vector.tensor_copy(out=aT_sb[h], in_=aTps)
        for h in range(HEADS):
            nc.tensor.matmul(askp[:, 48 * h:48 * (h + 1)], lhsT=aT_sb[h],
                             rhs=dgrk[h], start=True, stop=True,
                             skip_group_check=True)

        # ==== dw2 (PE diag taps) ===========================================
        for m in range(3):
            for cchunk in range(NCH):
                ps = psC.tile([128, CH], F32, name="cps2", tag="cps")
                dw_taps(ps, convx[m][:, CH * cchunk:CH * (cchunk + 1)],
                        list(range(9)), dw2_diag[m],
                        lambda t: fcol("dw2c", 9 * m + t, p=128),
                        lambda t: _win(conv1p[m],
                                       (8 * cchunk + t // 3) * HP + t % 3,
                                       [[HP, 8], [1, W]]),
                        dw2_b[m], 6)

    # ==== softmax (Vector/Scalar; overlaps sp-front PE work) ===============
    with tc.tile_pool(name="smx", bufs=2) as smx:
        scc = 0
        nmx_c = {}
        for (h, vt, off, klo, khi) in PIECES:
            w = khi - klo
            if klo == 0:
                ask = askp[:, 48 * h:48 * (h + 1)]
                mx = smx.tile([48, 1], F32, name="mx", tag="mx")
                nc.vector.tensor_reduce(out=mx, in_=ask, axis=AX.X,
                                        op=OP.max)
                nmx = smx.tile([48, 1], F32, name="nmx", tag=f"nmx{h}",
                               bufs=1)
                nc.vector.tensor_scalar(out=nmx, in0=mx,
                                        scalar1=rqs[:, h:h + 1],
                                        scalar2=-1.0,
                                        op0=OP.mult, op1=OP.mult)
                nmx_c[h] = nmx
            if h in (2, 5):
                acc = ssc[:, scc:scc + 1]
                scc += 1
            else:
                acc = ssum8[:, h:h + 1]
            nc.scalar.activation(
                out=en_vt[vt][:, off:off + w],
                in_=askp[:, 48 * h + klo:48 * h + khi],
                func=AF.Exp, bias=nmx_c[h], scale=rqs[:, h:h + 1],
                accum_out=acc)
        nc.vector.tensor_tensor(out=ssum8[:, 2:3], in0=ssc[:, 0:1],
                                in1=ssc[:, 1:2], op=OP.add)
        nc.vector.tensor_tensor(out=ssum8[:, 5:6], in0=ssc[:, 2:3],
                                in1=ssc[:, 3:4], op=OP.add)
        nc.vector.reciprocal(out=rs, in_=ssum8)
        for (h, vt, off, klo, khi) in PIECES:
            sli = en_vt[vt][:, off:off + (khi - klo)]
            nc.vector.tensor_scalar_mul(sli, sli, rs[:, h:h + 1])
        # transpose each en_vt -> SBUF, scatter blocks into E matrices via
        # SBUF->SBUF DMA (compute engines need 32-aligned partition bases;
        # DMA has no such restriction)
        pstS = [smx.tile([128, 48], BF, name=f"pstS{v}", tag=f"pstS{v}",
                         bufs=1) for v in range(3)]
        for vt in range(3):
            pvt = psT2.tile([128, 48], BF, name=f"pst{vt}", tag="t")
            nc.tensor.transpose(pvt, en_vt[vt], i48b)
            if vt % 2 == 0:
                nc.vector.tensor_copy(out=pstS[vt], in_=pvt)
            else:
                nc.scalar.copy(out=pstS[vt], in_=pvt)
        for (vt, gq, plo, phi, coff, clo, chi) in ECOPY:
            sync.dma_start(
                out=E[(vt, gq)][plo:phi, coff:coff + (chi - clo)],
                in_=pstS[vt][plo:phi, clo:chi])
    smstack.close()
    cvstack.close()  # conv1p dies after dw2

    # RIGHT: sp_pad
    spstack = ExitStack()
    sppool = spstack.enter_context(tc.tile_pool(name="spp", bufs=1,
                                                side="right"))
    sp_pad = [sppool.tile([96, NP1], BF, name=f"sppad{m}") for m in range(2)]
    for m in range(2):
        pad_borders(sp_pad[m], 96, H + 2, HP, 1)
    spdw_diag = [[mkdiag(wp, 96, fcol("spdwc", 9 * m + t_))
                 for t_ in range(9)] for m in range(2)]
    psEstack = ExitStack()
    psE = psEstack.enter_context(tc.tile_pool(name="psE", bufs=2,
                                              space="PSUM"))

    # ==== SpatialProjection front (PE; overlaps softmax) ===================
    for m in range(2):
        for cchunk in range(NCH):
            ps = psE.tile([96, CH], F32, name="eps", tag="eps")
            for k in range(3):
                nc.tensor.matmul(
                    ps, lhsT=spinT[k][:, 96 * m:96 * (m + 1)],
                    rhs=convx[k][:, CH * cchunk:CH * (cchunk + 1)],
                    start=(k == 0), stop=(k == 2))
            dst = _win(sp_pad[m], (8 * cchunk + 1) * HP + 1,
                       [[HP, 8], [1, W]])
            evac(dst, ps, sp_in_b[m])
    for m in range(2):
        for cchunk in range(NCH):
            ps = psE.tile([96, CH], F32, name="eps2", tag="eps")
            dw_taps(ps, dd[m][:, CH * cchunk:CH * (cchunk + 1)],
                    list(range(9)), spdw_diag[m],
                    lambda t: fcol("spdwc", 9 * m + t),
                    lambda t: _win(sp_pad[m],
                                   (8 * cchunk + t // 3) * HP + t % 3,
                                   [[HP, 8], [1, W]]),
                    sp_dw_b[m], 7)
    # gg = gelu(x1)*x2, chunked: gelu on scalar, mult on vector
    gg = dd[0]
    for q in range(4):
        sl = slice(1024 * q, 1024 * (q + 1))
        nc.scalar.activation(out=dd[0][:, sl], in_=dd[0][:, sl], func=AF.Gelu)
        nc.vector.tensor_mul(dd[0][:, sl], dd[0][:, sl], dd[1][:, sl])

    # ==== at = attn @ v  (128-channel groups via sparse E matrices) ========
    with tc.tile_pool(name="psAT", bufs=6, space="PSUM") as psAT:
        for cchunk in range(NCH):
            for gq in range(3):
                vts = EPAIRS[gq]
                ps = psAT.tile([128, CH], F32, name="atps", tag="atps")
                for ii, vt in enumerate(vts):
                    nc.tensor.matmul(
                        ps, lhsT=E[(vt, gq)],
                        rhs=v_sb[vt][:, CH * cchunk:CH * (cchunk + 1)],
                        start=(ii == 0), stop=(ii == len(vts) - 1))
                evac(at_sb[gq][:, CH * cchunk:CH * (cchunk + 1)], ps)
    spstack.close()  # sp_pad
    vstack.close()   # v

    # RIGHT: ChannelProjection chain buffers
    cpstack = ExitStack()
    tp = cpstack.enter_context(tc.tile_pool(name="tp", bufs=1, side="right"))
    psD = cpstack.enter_context(tc.tile_pool(name="psD", bufs=4, space="PSUM"))
    tpk = tp.tile([128, 34 * HP], BF, name="tpk")
    pad_borders(tpk, 128, 34, HP, 1)
    cia_diag = [mkdiag(wp, 128, fcol("ciac", t_, p=128)) for t_ in range(9)]
    cbp = tp.tile([128, 50 * BP], BF, name="cbp")
    pad_borders(cbp, 128, 50, BP, 9)
    cib_diag = [mkdiag(wp, 128, fcol("cibc", t_, p=128)) for t_ in range(49)]
    t_dense = tp.tile([C6, N], BF, name="t_dense")
    ci2v = tp.tile([C6, N], BF, name="ci2v")
    pprod = tp.tile([C6, N], BF, name="pprod")

    t_img = t_dense.rearrange("p (h w) -> p h w", w=W)
    for cchunk in range(NCH):
        ps = psD.tile([C6, CH], F32, name="dps", tag="dps")
        for k in range(3):
            nc.tensor.matmul(
                ps, lhsT=cpinT[k],
                rhs=at_sb[k][:, CH * cchunk:CH * (cchunk + 1)],
                start=(k == 0), stop=(k == 2))
        evac(t_dense[:, CH * cchunk:CH * (cchunk + 1)], ps, cp_in_b,
             accum=tsum[:, cchunk:cchunk + 1])
        # stream freshly-written t rows into the packed pad buffer
        r0, r1 = 8 * cchunk, 8 * cchunk + 8
        lo, hi = max(r0, 0), min(r1, 33)
        if lo < hi:
            sync.dma_start(
                out=_win(tpk, (lo + 1) * HP + 1, [[HP, hi - lo], [1, W]],
                         p=(0, 64)),
                in_=t_img[:, lo:hi, :])
        lo, hi = max(r0, 31), min(r1, 64)
        if lo < hi:
            sync.dma_start(
                out=_win(tpk, (lo - 31) * HP + 1, [[HP, hi - lo], [1, W]],
                         p=(64, 128)),
                in_=t_img[:, lo:hi, :])

    nc.vector.tensor_reduce(out=tm, in_=tsum, axis=AX.X, op=OP.add)
    nc.vector.tensor_scalar_mul(tm, tm, 1.0 / N)
    psc = psD.tile([C6, 1], F32, name="dps1", tag="dps")
    nc.tensor.matmul(psc, lhsT=ci1T, rhs=tm, start=True, stop=True)
    nc.vector.tensor_scalar(out=ci1v, in0=psc, scalar1=ci1_b,
                            scalar2=None, op0=OP.add)

    # ci2a (9 taps on PE) -> packed pad-9 buffer [128, 50*82]
    for cchunk in range(4):
        ps = psD.tile([128, CH], F32, name="dpsa", tag="dps")
        for t_ in range(9):
            dy, dx = divmod(t_, 3)
            src = _win(tpk, (8 * cchunk + dy) * HP + dx,
                       [[HP, 8], [1, W]])
            nc.tensor.matmul(ps, lhsT=cia_diag[t_], rhs=src,
                             start=(t_ == 0), stop=(t_ == 8))
        dst = _win(cbp, (8 * cchunk + 9) * BP + 9, [[BP, 8], [1, W]])
        evac(dst, ps, ci2a_b2)
    # halo exchange between halves
    sync.dma_start(
        out=_win(cbp, 41 * BP + 9, [[BP, 9], [1, W]], p=(0, 64)),
        in_=_win(cbp, 9 * BP + 9, [[BP, 9], [1, W]], p=(64, 128)))
    sync.dma_start(
        out=_win(cbp, 9, [[BP, 9], [1, W]], p=(64, 128)),
        in_=_win(cbp, 32 * BP + 9, [[BP, 9], [1, W]], p=(0, 64)))

    # ci2b 49 dilated taps on PE -> ci2c per chunk -> ci2v dense [64, N]
    for cchunk in range(4):
        ps = psD.tile([128, CH], F32, name="dpsb", tag="dps")
        cbd = cbd_ring[cchunk % 2]
        dw_taps(ps, cbd, list(range(49)), cib_diag,
                lambda t: fcol("cibc", t, p=128),
                lambda t: _win(cbp,
                               (8 * cchunk + 3 * (t // 7)) * BP
                               + 3 * (t % 7),
                               [[BP, 8], [1, W]]),
                ci2b_b2, 37)
        for half in range(2):
            ps2 = psD.tile([C6, CH], F32, name="dpsc", tag="dps")
            nc.tensor.matmul(ps2, lhsT=ci2cT[half], rhs=cbd,
                             start=True, stop=True)
            evac(ci2v[:, 2048 * half + CH * cchunk:
                      2048 * half + CH * (cchunk + 1)], ps2, ci2c_b)

    # pprod = t * ci1 * ci2 (chunked, accumulate mean for cm gate)
    for cchunk in range(NCH):
        sl = slice(CH * cchunk, CH * (cchunk + 1))
        nc.vector.scalar_tensor_tensor(
            out=pprod[:, sl], in0=t_dense[:, sl], scalar=ci1v,
            in1=ci2v[:, sl], op0=OP.mult, op1=OP.mult,
            accum_out=pm[:, cchunk:cchunk + 1])

    # cp_out fused into at; sp_out gate per chunk
    for cchunk in range(NCH):
        sl = slice(CH * cchunk, CH * (cchunk + 1))
        for m in range(3):
            ps = psD.tile([128, CH], F32, name="dpso", tag="dpso", bufs=2)
            nc.tensor.matmul(
                ps, lhsT=cpoutT[:, 128 * m:128 * (m + 1)],
                rhs=pprod[:, sl], start=True, stop=True)
            nc.vector.scalar_tensor_tensor(
                out=at_sb[m][:, sl], in0=ps, scalar=cp_out_b[m],
                in1=at_sb[m][:, sl], op0=OP.add, op1=OP.add)
        for m in range(3):
            ps = psE.tile([128, CH], F32, name="eps3", tag="eps")
            nc.tensor.matmul(
                ps, lhsT=spoutT[:, 128 * m:128 * (m + 1)],
                rhs=gg[:, sl], start=True, stop=True)
            sg = sg_ring[(3 * cchunk + m) % 3]
            nc.scalar.activation(out=sg, in_=ps, func=AF.Sigmoid,
                                 bias=sp_out_b[m], scale=1.0)
            nc.vector.tensor_mul(at_sb[m][:, sl], at_sb[m][:, sl], sg)

    # cm gate from mean(pprod) via linearity of cp_out
    nc.vector.tensor_reduce(out=pmean32, in_=pm, axis=AX.X, op=OP.add)
    nc.vector.tensor_scalar_mul(pmean, pmean32, 1.0 / N)
    for m in range(3):
        ps = psD.tile([128, 1], F32, name="dpsm", tag="dpso", bufs=2)
        nc.tensor.matmul(ps, lhsT=cpoutT[:, 128 * m:128 * (m + 1)],
                         rhs=pmean, start=True, stop=True)
        nc.scalar.activation(out=cm_sig[:, m:m + 1], in_=ps, func=AF.Sigmoid,
                             bias=cp_out_b[m], scale=1.0)
    cpstack.close()
    psEstack.close()

    # ==== final gates + proj + store =======================================
    with tc.tile_pool(name="psF", bufs=6, space="PSUM") as psF:
        for cchunk in range(NCH):
            sl = slice(CH * cchunk, CH * (cchunk + 1))
            for m in range(3):
                nc.vector.scalar_tensor_tensor(
                    out=at_sb[m][:, sl], in0=convx[m][:, sl],
                    scalar=cm_sig[:, m:m + 1], in1=at_sb[m][:, sl],
                    op0=OP.mult, op1=OP.add)
            for m in range(3):
                ps = psF.tile([128, CH], F32, name="fps", tag="fps")
                for k in range(3):
                    nc.tensor.matmul(
                        ps, lhsT=projT[k][:, 128 * m:128 * (m + 1)],
                        rhs=at_sb[k][:, sl],
                        start=(k == 0), stop=(k == 2))
                ot = oring[(3 * cchunk + m) % 4]
                evac(ot, ps, proj_b[m])
                sync.dma_start(
                    out=d["out"][128 * m:128 * (m + 1), sl], in_=ot)


def build_program():
    nc = bacc.Bacc("TRN2", target_bir_lowering=False, debug=False,
                   num_devices=NCORES)
    d = {}
    d["wt"] = nc.dram_tensor("wt", [3, 128, 3 * C], BF, kind="ExternalInput")
    d["blob16"] = nc.dram_tensor("blob16", [128, BCOLS], BF,
                                 kind="ExternalInput")
    d["blob32"] = nc.dram_tensor("blob32", [128, FCOLS], F32,
                                 kind="ExternalInput")
    d["x"] = nc.dram_tensor("x", [C, N], BF, kind="ExternalInput")
    d["out"] = nc.dram_tensor("out", [C, N], F32, kind="ExternalOutput")

    with tile.TileContext(nc) as tc:
        emit(tc, d)
    nc.compile()
    return nc


_cached = None


def kernel(**inputs) -> np.ndarray:
    global _last_results, _cached
    x = np.asarray(inputs["x"], np.float32)
    B = x.shape[0]
    assert x.shape == (NCORES, C, H, W), x.shape
    g = build_host_inputs(inputs)
    if _cached is None:
        _cached = build_program()
    nc = _cached

    base = {nm: np.ascontiguousarray(arr) for nm, arr in g.items()}
    in_maps = []
    for b in range(B):
        m = dict(base)
        m["x"] = np.ascontiguousarray(x[b].reshape(C, N)).astype(BF16)
        in_maps.append(m)

    trace = os.environ.get("KERNEL_TRACE") == "1"
    try:
        res = run_bass_kernel_spmd(nc, in_maps, list(range(NCORES)),
                                   trace=trace)
    except ModuleNotFoundError:
        res = run_bass_kernel_spmd(nc, in_maps, list(range(NCORES)),
                                   trace=False)
    _last_results = res
    out = np.stack([res.results[b]["out"].reshape(C, H, W) for b in range(B)])
    return out.astype(np.float32)


# revision 19
# speedup vs baseline: 1.2064x; 1.2064x over previous
"""Trainium2 Bass kernel for nn_Channel_Transposed_Attention (B8 C384 H64 W64).

Data-parallel over batch: 8 batch elements -> 8 NeuronCores (SPMD, per-core
x slice). Per core everything lives in (C, N) channel-major layout (N=H*W).
Channel convs/projections use 128-channel tiles (3 per C=384); attention
head structure (8 heads x 48) uses 48/96-partition tiles. q,k are produced
in (N, C) token-major layout; per head the [q_h|k_h] Gram matrix gives both
the attention logits and the l2-norm diagonals in one accumulated matmul
chain. The scaled logits A*diag(rk) are computed on the PE via a per-head
transpose + diagonal matmul (rq folds into the exp scale), avoiding any
partition broadcast. The attention output at = attn @ v runs in 128-channel
tiles via 7 sparse block matrices E[vt,g] assembled from transposed softmax
blocks. Depthwise convs run as PE tap-accumulation with per-channel diagonal
weight matrices (bf16, built on-device) over zero-padded row-strided buffers
(only borders are zeroed). Weights ship as two packed blobs; x streams per
512-token chunk on the GpSimd DMA queue.
"""
import os
import numpy as np
from contextlib import ExitStack

import concourse.bass as bass
import concourse.bacc as bacc
import concourse.tile as tile
from concourse import mybir
from concourse.bass_utils import run_bass_kernel_spmd
from concourse._compat import with_exitstack

import ml_dtypes
BF16 = ml_dtypes.bfloat16

F32 = mybir.dt.float32
BF = mybir.dt.bfloat16
AF = mybir.ActivationFunctionType
OP = mybir.AluOpType
AX = mybir.AxisListType

H = W = 64
N = H * W               # 4096
HP = W + 2              # 66   pad-1 row stride
NP1 = (H + 2) * HP      # 4356
BP = W + 18             # 82   pad-9 row stride (ci2b)
C = 384
C6, C2, C4 = 64, 192, 96
HEADS, HD = 8, 48
NCORES = 8
CH = 512
NCH = N // CH           # 8
QKW = 2 * C             # 768

# softmax pieces: (head, v-tile, col-offset in en_vt, k_lo, k_hi)
PIECES = [(0, 0, 0, 0, 48), (1, 0, 48, 0, 48),
          (2, 0, 96, 0, 32), (2, 1, 0, 32, 48),
          (3, 1, 16, 0, 48), (4, 1, 64, 0, 48),
          (5, 1, 112, 0, 16), (5, 2, 0, 16, 48),
          (6, 2, 32, 0, 48), (7, 2, 80, 0, 48)]
# E-matrix assembly: (vt, g, prow_lo, prow_hi, col_off, c_lo, c_hi)
ECOPY = [(0, 0, 0, 48, 0, 0, 48),        # h0
         (0, 0, 48, 96, 48, 0, 48),      # h1
         (0, 0, 96, 128, 96, 0, 32),     # h2
         (0, 1, 96, 128, 0, 32, 48),
         (1, 0, 0, 16, 96, 0, 32),
         (1, 1, 0, 16, 0, 32, 48),
         (1, 1, 16, 64, 16, 0, 48),      # h3
         (1, 1, 64, 112, 64, 0, 48),     # h4
         (1, 1, 112, 128, 112, 0, 16),   # h5
         (1, 2, 112, 128, 0, 16, 48),
         (2, 1, 0, 32, 112, 0, 16),
         (2, 2, 0, 32, 0, 16, 48),
         (2, 2, 32, 80, 32, 0, 48),      # h6
         (2, 2, 80, 128, 80, 0, 48)]     # h7
EPAIRS = {0: [0, 1], 1: [0, 1, 2], 2: [1, 2]}

_last_results = None


def _win(t, off, dims, p=None):
    """Strided free-dim window of a 2D tile AP at free element offset."""
    base = t[:, off:off + 1] if p is None else t[p[0]:p[1], off:off + 1]
    return bass.AP(tensor=base.tensor, offset=base.offset,
                   ap=[list(base.ap[0])] + [list(dd) for dd in dims])


# ---- packed f32 blob column layout -----------------------------------------
FB = {}
_c = 0
def _fcol(name, n):
    global _c
    FB[name] = _c
    _c += n
_fcol("dw1_b", 3)      # [128] x3
_fcol("dw2_b", 3)
_fcol("cp_in_b", 1)    # [64]
_fcol("ci1_b", 1)
_fcol("ci2a_b2", 1)    # [128]
_fcol("ci2b_b2", 1)
_fcol("ci2c_b", 1)
_fcol("cp_out_b", 3)   # [128] x3
_fcol("sp_in_b", 2)    # [96] x2
_fcol("sp_dw_b", 2)
_fcol("sp_out_b", 3)   # [128] x3
_fcol("proj_b", 3)
_fcol("temp", 8)       # broadcast [96, 8]
_fcol("ci1T", 64)      # [64, 64]
_fcol("dw2c", 27)      # [128, 9] x3 m-major
_fcol("spdwc", 18)     # [96, 9] x2
_fcol("ciac", 9)       # [128, 9]
_fcol("cibc", 49)      # [128, 49]
FCOLS = _c

# ---- packed bf16 blob column layout ----------------------------------------
BB = {}
_c = 0
def _bcol(name, n):
    global _c
    BB[name] = _c
    _c += n
_bcol("dw1T", 3 * C)    # [128, 384] x3 k-tiles
_bcol("projT", 3 * C)   # [128, 384] x3
_bcol("cpinT", 3 * C6)  # [128, 64] x3
_bcol("ci2cT", 2 * C6)  # [128, 64] x2 (halves on partition ranges)
_bcol("cpoutT", C)      # [64, 384]
_bcol("spinT", 3 * C2)  # [128, 192] x3
_bcol("spoutT", C)      # [96, 384]
BCOLS = _c


def build_host_inputs(inputs):
    g = {}
    qkv_w = np.asarray(inputs["qkv_w"], np.float32)
    wt = qkv_w.T                                     # [384, 1152] qk | v
    g["wt"] = np.ascontiguousarray(wt.reshape(3, 128, 3 * C)).astype(BF16)

    bb = np.zeros((128, BCOLS), np.float32)
    def put3(nm, mat, w):
        for k in range(3):
            bb[:, BB[nm] + w * k:BB[nm] + w * (k + 1)] = \
                mat[128 * k:128 * (k + 1), :]
    put3("dw1T", np.asarray(inputs["dw1_w"], np.float32).reshape(C, C).T, C)
    put3("projT", np.asarray(inputs["proj_w"], np.float32).T, C)
    put3("cpinT", np.asarray(inputs["cp_in_w"], np.float32).reshape(C6, C).T,
         C6)
    put3("spinT", np.asarray(inputs["sp_in_w"], np.float32).reshape(C2, C).T,
         C2)
    ci2cT = np.asarray(inputs["ci2c_w"], np.float32).reshape(C6, C6).T
    bb[0:64, BB["ci2cT"]:BB["ci2cT"] + C6] = ci2cT
    bb[64:128, BB["ci2cT"] + C6:BB["ci2cT"] + 2 * C6] = ci2cT
    bb[0:64, BB["cpoutT"]:BB["cpoutT"] + C] = \
        np.asarray(inputs["cp_out_w"], np.float32).reshape(C, C6).T
    bb[0:96, BB["spoutT"]:BB["spoutT"] + C] = \
        np.asarray(inputs["sp_out_w"], np.float32).reshape(C, C4).T
    g["blob16"] = np.ascontiguousarray(bb).astype(BF16)

    fb = np.zeros((128, FCOLS), np.float32)
    def put(nm, vec, p=None):
        vec = np.asarray(vec, np.float32).reshape(-1)
        fb[0:len(vec), FB[nm] + (p or 0)] = vec
    for m in range(3):
        put("dw1_b", inputs["dw1_b"][128 * m:128 * (m + 1)], m)
        put("dw2_b", inputs["dw2_b"][128 * m:128 * (m + 1)], m)
        put("cp_out_b", inputs["cp_out_b"][128 * m:128 * (m + 1)], m)
        put("sp_out_b", inputs["sp_out_b"][128 * m:128 * (m + 1)], m)
        put("proj_b", inputs["proj_b"][128 * m:128 * (m + 1)], m)
    for m in range(2):
        put("sp_in_b", inputs["sp_in_b"][96 * m:96 * (m + 1)], m)
        put("sp_dw_b", inputs["sp_dw_b"][96 * m:96 * (m + 1)], m)
    put("cp_in_b", inputs["cp_in_b"])
    put("ci1_b", inputs["ci1_b"])
    put("ci2a_b2", np.tile(np.asarray(inputs["ci2a_b"], np.float32), 2))
    put("ci2b_b2", np.tile(np.asarray(inputs["ci2b_b"], np.float32), 2))
    put("ci2c_b", inputs["ci2c_b"])
    temp = np.asarray(inputs["temperature"], np.float32).reshape(1, HEADS)
    fb[0:96, FB["temp"]:FB["temp"] + 8] = np.broadcast_to(temp, (96, 8))
    fb[0:64, FB["ci1T"]:FB["ci1T"] + 64] = \
        np.asarray(inputs["ci1_w"], np.float32).reshape(C6, C6).T
    dw2 = np.asarray(inputs["dw2_w"], np.float32).reshape(C, 9)
    for m in range(3):
        fb[:, FB["dw2c"] + 9 * m:FB["dw2c"] + 9 * (m + 1)] = \
            dw2[128 * m:128 * (m + 1)]
    spdw = np.asarray(inputs["sp_dw_w"], np.float32).reshape(C2, 9)
    for m in range(2):
        fb[0:96, FB["spdwc"] + 9 * m:FB["spdwc"] + 9 * (m + 1)] = \
            spdw[96 * m:96 * (m + 1)]
    cia = np.asarray(inputs["ci2a_w"], np.float32).reshape(C6, 9)
    fb[:, FB["ciac"]:FB["ciac"] + 9] = np.vstack([cia, cia])
    cib = np.asarray(inputs["ci2b_w"], np.float32).reshape(C6, 49)
    fb[:, FB["cibc"]:FB["cibc"] + 49] = np.vstack([cib, cib])
    g["blob32"] = np.ascontiguousarray(fb)
    return g


@with_exitstack
def emit(ctx: ExitStack, tc, d):
    nc = tc.nc
    sync = nc.sync
    gp = nc.gpsimd

    # ---- LEFT: one persistent pool, all tiles created up front -------------
    wp = ctx.enter_context(tc.tile_pool(name="wp", bufs=1, side="left"))

    # RIGHT stack bottom: v (phase1 -> at)
    vstack = ExitStack()
    vp = vstack.enter_context(tc.tile_pool(name="vp", bufs=1, side="right"))
    v_sb = [vp.tile([128, N], BF, name=f"v{m}") for m in range(3)]

    # RIGHT: phase-1 transients (wt + x ring + qkt ring)
    p1stack = ExitStack()
    xw = p1stack.enter_context(tc.tile_pool(name="xw", bufs=1, side="right"))
    xring = p1stack.enter_context(tc.tile_pool(name="xring", bufs=3,
                                               side="right"))
    qkring = p1stack.enter_context(tc.tile_pool(name="qkring", bufs=3,
                                                side="right"))
    wt_sb = [xw.tile([128, 3 * C], BF, name=f"wt{k}") for k in range(3)]
    for k in range(3):
        sync.dma_start(out=wt_sb[k], in_=d["wt"][k])

    blob16 = wp.tile([128, BCOLS], BF, name="blob16")
    blob32 = wp.tile([128, FCOLS], F32, name="blob32")

    def bview(nm, i, w, p=128):
        return blob16[0:p, BB[nm] + w * i:BB[nm] + w * (i + 1)]

    def fcol(nm, i=0, p=96):
        return blob32[0:p, FB[nm] + i:FB[nm] + i + 1]

    dw1T = [bview("dw1T", k, C) for k in range(3)]
    projT = [bview("projT", k, C) for k in range(3)]
    cpinT = [bview("cpinT", k, C6) for k in range(3)]
    ci2cT = [bview("ci2cT", h, C6) for h in range(2)]
    cpoutT = bview("cpoutT", 0, C, 64)
    spinT = [bview("spinT", k, C2) for k in range(3)]
    spoutT = bview("spoutT", 0, C, 96)
    ci1T = blob32[0:64, FB["ci1T"]:FB["ci1T"] + 64]
    tempb = blob32[0:96, FB["temp"]:FB["temp"] + 8]
    dw1_b = [fcol("dw1_b", m, p=128) for m in range(3)]
    dw2_b = [fcol("dw2_b", m, p=128) for m in range(3)]
    cp_in_b = fcol("cp_in_b", p=64)
    ci1_b = fcol("ci1_b", p=64)
    ci2a_b2 = fcol("ci2a_b2", p=128)
    ci2b_b2 = fcol("ci2b_b2", p=128)
    ci2c_b = fcol("ci2c_b", p=64)
    cp_out_b = [fcol("cp_out_b", m, p=128) for m in range(3)]
    sp_in_b = [fcol("sp_in_b", m) for m in range(2)]
    sp_dw_b = [fcol("sp_dw_b", m) for m in range(2)]
    sp_out_b = [fcol("sp_out_b", m, p=128) for m in range(3)]
    proj_b = [fcol("proj_b", m, p=128) for m in range(3)]

    # LEFT persistents
    i128 = wp.tile([128, 128], F32, name="i128")
    i48b = wp.tile([48, 48], BF, name="i48b")
    ssq = wp.tile([96, HEADS], F32, name="ssq")
    gscr = wp.tile([96, 96], BF, name="gscr")
    rn = wp.tile([96, HEADS], F32, name="rn")
    rnT = wp.tile([HEADS, 96], F32, name="rnT")
    rqs = wp.tile([48, HEADS], F32, name="rqs")
    ssc = wp.tile([48, 4], F32, name="ssc")
    ssum8 = wp.tile([48, HEADS], F32, name="ssum8")
    rs = wp.tile([48, HEADS], F32, name="rs")
    a_sb = [wp.tile([48, 48], F32, name=f"a{h}") for h in range(HEADS)]
    aT_sb = [wp.tile([48, 48], F32, name=f"aT{h}") for h in range(HEADS)]
    dgrk = [wp.tile([48, 48], F32, name=f"dgrk{h}") for h in range(HEADS)]
    rkcol = wp.tile([48, HEADS], F32, name="rkcol")
    en_vt = [wp.tile([48, 128], BF, name=f"envt{v}") for v in range(3)]
    E = {}
    for (vt, gq) in [(0, 0), (0, 1), (1, 0), (1, 1), (1, 2), (2, 1), (2, 2)]:
        E[(vt, gq)] = wp.tile([128, 128], BF, name=f"E{vt}{gq}")
    cm_sig = wp.tile([128, 3], F32, name="cm_sig")
    tsum = wp.tile([C6, NCH], F32, name="tsum")
    tm = wp.tile([C6, 1], F32, name="tm")
    ci1v = wp.tile([C6, 1], F32, name="ci1v")
    pm = wp.tile([C6, NCH], F32, name="pm")
    pmean32 = wp.tile([C6, 1], F32, name="pmean32")
    pmean = wp.tile([C6, 1], BF, name="pmean")
    at_sb = [wp.tile([128, N], BF, name=f"at{m}") for m in range(3)]
    convx = [wp.tile([128, N], BF, name=f"cx{m}") for m in range(3)]
    dd = [wp.tile([96, N], BF, name=f"dd{m}") for m in range(2)]
    sg_ring = [wp.tile([128, CH], BF, name=f"sg{i}") for i in range(3)]
    cbd_ring = [wp.tile([128, CH], BF, name=f"cbd{i}") for i in range(2)]
    oring = [wp.tile([128, CH], F32, name=f"oring{i}") for i in range(4)]
    dacc = [wp.tile([128, CH], BF, name=f"dacc{i}") for i in range(3)]

    # startup on-device constants
    gp.memset(i128, 1.0)
    gp.affine_select(out=i128, in_=i128, pattern=[[-1, 128]], base=0,
                     channel_multiplier=1, compare_op=OP.is_equal, fill=0.0)
    i96 = i128[0:96, 0:96]
    gp.tensor_copy(out=i48b, in_=i128[0:48, 0:48])
    for e in E.values():
        gp.memset(e, 0.0)

    _dgi = [0]

    def mkdiag(pool, p, col):
        _dgi[0] += 1
        t = pool.tile([p, p], BF, name=f"dg{_dgi[0]}")
        if _dgi[0] % 2 == 0:
            nc.vector.tensor_scalar(out=t, in0=i128[0:p, 0:p], scalar1=col,
                                    scalar2=None, op0=OP.mult)
        else:
            nc.scalar.activation(out=t, in_=i128[0:p, 0:p], func=AF.Copy,
                                 scale=col)
        return t

    def pad_borders(t, p, nrow, stride, pw):
        """Zero only the pad borders of an image buffer."""
        eng = nc.vector
        eng.memset(_win(t, 0, [[1, pw * stride]], p=(0, p)), 0.0)
        eng.memset(_win(t, (nrow - pw) * stride, [[1, pw * stride]],
                        p=(0, p)), 0.0)
        eng.memset(_win(t, pw * stride, [[stride, nrow - 2 * pw],
                                         [stride - pw, 2], [1, pw]],
                        p=(0, p)), 0.0)


    _dri = [0]

    def dw_taps(ps, out, taps, diag, wcol_f, src_f, bias, pe_taps):
        """Depthwise conv: first pe_taps taps on PE (PSUM), rest chained on
        Vector; final out = (ps + bias) + acc."""
        for ti in range(pe_taps):
            nc.tensor.matmul(ps, lhsT=diag[taps[ti]], rhs=src_f(taps[ti]),
                             start=(ti == 0), stop=(ti == pe_taps - 1))
        _dri[0] += 1
        acc = dacc[_dri[0] % 3]
        p = ps.partition_size()
        for i, t_ in enumerate(taps[pe_taps:]):
            if i == 0:
                nc.vector.tensor_scalar(out=acc[0:p, :], in0=src_f(t_),
                                        scalar1=wcol_f(t_), scalar2=None,
                                        op0=OP.mult)
            else:
                nc.vector.scalar_tensor_tensor(
                    out=acc[0:p, :], in0=src_f(t_), scalar=wcol_f(t_),
                    in1=acc[0:p, :], op0=OP.mult, op1=OP.add)
        nc.vector.scalar_tensor_tensor(out=out, in0=ps, scalar=bias,
                                       in1=acc[0:p, :], op0=OP.add,
                                       op1=OP.add)

    # evacuation helper: alternate Vector / Scalar engines
    _evi = [0]

    def evac(out, ps, bias=None, accum=None):
        _evi[0] += 1
        if _evi[0] % 3 == 0:
            if bias is None:
                nc.vector.tensor_copy(out=out, in_=ps)
            elif accum is None:
                nc.vector.tensor_scalar(out=out, in0=ps, scalar1=bias,
                                        scalar2=None, op0=OP.add)
            else:
                nc.vector.tensor_scalar(out=out, in0=ps, scalar1=bias,
                                        scalar2=0.0, op0=OP.add, op1=OP.add,
                                        accum_out=accum)
        else:
            if bias is None:
                nc.scalar.copy(out=out, in_=ps)
            else:
                nc.scalar.activation(out=out, in_=ps, func=AF.Identity,
                                     bias=bias, scale=1.0, accum_out=accum)

    # ==== phase 1: qkv + head Grams ========================================
    with tc.tile_pool(name="psQK", bufs=2, space="PSUM") as psQK, \
         tc.tile_pool(name="psV", bufs=2, space="PSUM") as psV, \
         tc.tile_pool(name="psG", bufs=1, space="PSUM") as psG:
        gps = psG.tile([96, HEADS * 96], F32, name="gps")
        for cchunk in range(NCH):
            xc = [xring.tile([128, CH], BF, name=f"xc{k}", tag=f"xc{k}")
                  for k in range(3)]
            for k in range(3):
                nc.scalar.dma_start(
                    out=xc[k], in_=d["x"][128 * k:128 * (k + 1),
                                          CH * cchunk:CH * (cchunk + 1)])
            if cchunk == 1:
                sync.dma_start(out=blob16, in_=d["blob16"][:])
                sync.dma_start(out=blob32, in_=d["blob32"][:])
            for j in range(4):
                i = 4 * cchunk + j
                ps = psQK.tile([128, QKW], F32, name="qkps", tag="qkps")
                for o0, ow in ((0, 512), (512, 256)):
                    for k in range(3):
                        nc.tensor.matmul(
                            ps[:, o0:o0 + ow],
                            lhsT=xc[k][:, 128 * j:128 * (j + 1)],
                            rhs=wt_sb[k][:, o0:o0 + ow],
                            start=(k == 0), stop=(k == 2))
                # store head-interleaved: [h0: q48|k48][h1: q48|k48]...
                qkt = qkring.tile([128, QKW], BF, name="qkt", tag="qkt")
                dst = qkt.rearrange("p (h two f) -> p two h f",
                                    two=2, h=HEADS, f=HD)
                srcv = ps.rearrange("p (two h f) -> p two h f",
                                    two=2, h=HEADS, f=HD)
                if i % 2 == 0:
                    nc.scalar.copy(out=dst, in_=srcv)
                else:
                    nc.vector.tensor_copy(out=dst, in_=srcv)
                for h in range(HEADS):
                    lap = qkt[:, 96 * h:96 * (h + 1)]
                    nc.tensor.matmul(
                        gps[:, 96 * h:96 * (h + 1)], lhsT=lap, rhs=lap,
                        start=(i == 0), stop=(i == 4 * NCH - 1),
                        skip_group_check=True)
            for m in range(3):
                ps = psV.tile([128, CH], F32, name="vps", tag="vps")
                for k in range(3):
                    nc.tensor.matmul(
                        ps,
                        lhsT=wt_sb[k][:, QKW + 128 * m:QKW + 128 * (m + 1)],
                        rhs=xc[k],
                        start=(k == 0), stop=(k == 2))
                nc.scalar.copy(
                    out=v_sb[m][:, CH * cchunk:CH * (cchunk + 1)], in_=ps)

        # per-head: diag -> ssq column; copy A block to SBUF
        for h in range(HEADS):
            nc.vector.scalar_tensor_tensor(
                out=gscr, in0=gps[:, 96 * h:96 * (h + 1)], scalar=1.0,
                in1=i96, op0=OP.mult, op1=OP.mult,
                accum_out=ssq[:, h:h + 1])
            nc.vector.tensor_copy(
                out=a_sb[h], in_=gps[0:48, 96 * h + 48:96 * h + 96])
    p1stack.close()   # free wt + x ring + qkt ring

    # softmax prep on Vector/Scalar (runs under dw1 PE work)
    nc.scalar.activation(out=rn, in_=ssq, func=AF.Sqrt)
    nc.vector.reciprocal(out=rn, in_=rn)
    nc.vector.tensor_mul(rqs, rn[0:48, :], tempb[0:48, :])

    # RIGHT: conv1p pads; dw2 diags live in wp
    cvstack = ExitStack()
    c1pool = cvstack.enter_context(tc.tile_pool(name="c1p", bufs=1,
                                                side="right"))
    conv1p = [c1pool.tile([128, NP1], BF, name=f"c1p{m}") for m in range(3)]
    for m in range(3):
        pad_borders(conv1p[m], 128, H + 2, HP, 1)
    dw2_diag = [[mkdiag(wp, 128, fcol("dw2c", 9 * m + t_, p=128))
                 for t_ in range(9)] for m in range(3)]

    smstack = ExitStack()
    psB = smstack.enter_context(tc.tile_pool(name="psB", bufs=1, space="PSUM"))
    psT2 = smstack.enter_context(tc.tile_pool(name="psT2", bufs=3,
                                              space="PSUM"))
    askp = psB.tile([48, 48 * HEADS], F32, name="askp")

    # ==== dw1 (PE) =========================================================
    with tc.tile_pool(name="psC", bufs=4, space="PSUM") as psC:
        for m in range(3):
            for cchunk in range(NCH):
                ps = psC.tile([128, CH], F32, name="cps", tag="cps")
                for k in range(3):
                    nc.tensor.matmul(
                        ps, lhsT=dw1T[k][:, 128 * m:128 * (m + 1)],
                        rhs=v_sb[k][:, CH * cchunk:CH * (cchunk + 1)],
                        start=(k == 0), stop=(k == 2))
                dst = _win(conv1p[m], (8 * cchunk + 1) * HP + 1,
                           [[HP, 8], [1, W]])
                evac(dst, ps, dw1_b[m])

        # softmax-prep transposes + scaled logits (PE, after dw1)
        psr = psT2.tile([HEADS, 96], F32, name="psr", tag="t")
        nc.tensor.transpose(psr, rn, i96)
        nc.vector.tensor_copy(out=rnT, in_=psr)
        pkc = psT2.tile([48, HEADS], F32, name="pkc", tag="t")
        nc.tensor.transpose(pkc, rnT[0:8, 48:96], i128[0:8, 0:8])
        nc.vector.tensor_copy(out=rkcol, in_=pkc)
        for h in range(HEADS):
            nc.vector.tensor_scalar(out=dgrk[h], in0=i128[0:48, 0:48],
                                    scalar1=rkcol[:, h:h + 1], scalar2=None,
                                    op0=OP.mult)
        for h in range(HEADS):
            aTps = psT2.tile([48, 48], F32, name="aTps", tag="t")
            nc.tensor.transpose(aTps, a_sb[h], i128[0:48, 0:48])
            nc.vector.tensor_copy(out=aT_sb[h], in_=aTps)
        for h in range(HEADS):
            nc.tensor.matmul(askp[:, 48 * h:48 * (h + 1)], lhsT=aT_sb[h],
                             rhs=dgrk[h], start=True, stop=True,
                             skip_group_check=True)

        # ==== dw2 (PE diag taps) ===========================================
        for m in range(3):
            for cchunk in range(NCH):
                ps = psC.tile([128, CH], F32, name="cps2", tag="cps")
                for t_ in range(9):
                    dy, dx = divmod(t_, 3)
                    src = _win(conv1p[m], (8 * cchunk + dy) * HP + dx,
                               [[HP, 8], [1, W]])
                    nc.tensor.matmul(ps, lhsT=dw2_diag[m][t_], rhs=src,
                                     start=(t_ == 0), stop=(t_ == 8))
                evac(convx[m][:, CH * cchunk:CH * (cchunk + 1)], ps, dw2_b[m])

    # ==== softmax (Vector/Scalar; overlaps sp-front PE work) ===============
    with tc.tile_pool(name="smx", bufs=2) as smx:
        scc = 0
        nmx_c = {}
        for (h, vt, off, klo, khi) in PIECES:
            w = khi - klo
            if klo == 0:
                ask = askp[:, 48 * h:48 * (h + 1)]
                mx = smx.tile([48, 1], F32, name="mx", tag="mx")
                nc.vector.tensor_reduce(out=mx, in_=ask, axis=AX.X,
                                        op=OP.max)
                nmx = smx.tile([48, 1], F32, name="nmx", tag=f"nmx{h}",
                               bufs=1)
                nc.vector.tensor_scalar(out=nmx, in0=mx,
                                        scalar1=rqs[:, h:h + 1],
                                        scalar2=-1.0,
                                        op0=OP.mult, op1=OP.mult)
                nmx_c[h] = nmx
            if h in (2, 5):
                acc = ssc[:, scc:scc + 1]
                scc += 1
            else:
                acc = ssum8[:, h:h + 1]
            nc.scalar.activation(
                out=en_vt[vt][:, off:off + w],
                in_=askp[:, 48 * h + klo:48 * h + khi],
                func=AF.Exp, bias=nmx_c[h], scale=rqs[:, h:h + 1],
                accum_out=acc)
        nc.vector.tensor_tensor(out=ssum8[:, 2:3], in0=ssc[:, 0:1],
                                in1=ssc[:, 1:2], op=OP.add)
        nc.vector.tensor_tensor(out=ssum8[:, 5:6], in0=ssc[:, 2:3],
                                in1=ssc[:, 3:4], op=OP.add)
        nc.vector.reciprocal(out=rs, in_=ssum8)
        for (h, vt, off, klo, khi) in PIECES:
            sli = en_vt[vt][:, off:off + (khi - klo)]
            nc.vector.tensor_scalar_mul(sli, sli, rs[:, h:h + 1])
        # transpose each en_vt -> SBUF, scatter blocks into E matrices via
        # SBUF->SBUF DMA (compute engines need 32-aligned partition bases;
        # DMA has no such restriction)
        pstS = [smx.tile([128, 48], BF, name=f"pstS{v}", tag=f"pstS{v}",
                         bufs=1) for v in range(3)]
        for vt in range(3):
            pvt = psT2.tile([128, 48], BF, name=f"pst{vt}", tag="t")
            nc.tensor.transpose(pvt, en_vt[vt], i48b)
            if vt % 2 == 0:
                nc.vector.tensor_copy(out=pstS[vt], in_=pvt)
            else:
                nc.scalar.copy(out=pstS[vt], in_=pvt)
        for (vt, gq, plo, phi, coff, clo, chi) in ECOPY:
            sync.dma_start(
                out=E[(vt, gq)][plo:phi, coff:coff + (chi - clo)],
                in_=pstS[vt][plo:phi, clo:chi])
    smstack.close()
    cvstack.close()  # conv1p dies after dw2

    # RIGHT: sp_pad
    spstack = ExitStack()
    sppool = spstack.enter_context(tc.tile_pool(name="spp", bufs=1,
                                                side="right"))
    sp_pad = [sppool.tile([96, NP1], BF, name=f"sppad{m}") for m in range(2)]
    for m in range(2):
        pad_borders(sp_pad[m], 96, H + 2, HP, 1)
    spdw_diag = [[mkdiag(wp, 96, fcol("spdwc", 9 * m + t_))
                 for t_ in range(9)] for m in range(2)]
    psEstack = ExitStack()
    psE = psEstack.enter_context(tc.tile_pool(name="psE", bufs=2,
                                              space="PSUM"))

    # ==== SpatialProjection front (PE; overlaps softmax) ===================
    for m in range(2):
        for cchunk in range(NCH):
            ps = psE.tile([96, CH], F32, name="eps", tag="eps")
            for k in range(3):
                nc.tensor.matmul(
                    ps, lhsT=spinT[k][:, 96 * m:96 * (m + 1)],
                    rhs=convx[k][:, CH * cchunk:CH * (cchunk + 1)],
                    start=(k == 0), stop=(k == 2))
            dst = _win(sp_pad[m], (8 * cchunk + 1) * HP + 1,
                       [[HP, 8], [1, W]])
            evac(dst, ps, sp_in_b[m])
    for m in range(2):
        for cchunk in range(NCH):
            ps = psE.tile([96, CH], F32, name="eps2", tag="eps")
            for t_ in range(9):
                dy, dx = divmod(t_, 3)
                src = _win(sp_pad[m], (8 * cchunk + dy) * HP + dx,
                           [[HP, 8], [1, W]])
                nc.tensor.matmul(ps, lhsT=spdw_diag[m][t_], rhs=src,
                                 start=(t_ == 0), stop=(t_ == 8))
            evac(dd[m][:, CH * cchunk:CH * (cchunk + 1)], ps, sp_dw_b[m])
    # gg = gelu(x1)*x2, chunked: gelu on scalar, mult on vector
    gg = dd[0]
    for q in range(4):
        sl = slice(1024 * q, 1024 * (q + 1))
        nc.scalar.activation(out=dd[0][:, sl], in_=dd[0][:, sl], func=AF.Gelu)
        nc.vector.tensor_mul(dd[0][:, sl], dd[0][:, sl], dd[1][:, sl])

    # ==== at = attn @ v  (128-channel groups via sparse E matrices) ========
    with tc.tile_pool(name="psAT", bufs=6, space="PSUM") as psAT:
        for cchunk in range(NCH):
            for gq in range(3):
                vts = EPAIRS[gq]
                ps = psAT.tile([128, CH], F32, name="atps", tag="atps")
                for ii, vt in enumerate(vts):
                    nc.tensor.matmul(
                        ps, lhsT=E[(vt, gq)],
                        rhs=v_sb[vt][:, CH * cchunk:CH * (cchunk + 1)],
                        start=(ii == 0), stop=(ii == len(vts) - 1))
                evac(at_sb[gq][:, CH * cchunk:CH * (cchunk + 1)], ps)
    spstack.close()  # sp_pad
    vstack.close()   # v

    # RIGHT: ChannelProjection chain buffers
    cpstack = ExitStack()
    tp = cpstack.enter_context(tc.tile_pool(name="tp", bufs=1, side="right"))
    psD = cpstack.enter_context(tc.tile_pool(name="psD", bufs=4, space="PSUM"))
    tpk = tp.tile([128, 34 * HP], BF, name="tpk")
    pad_borders(tpk, 128, 34, HP, 1)
    cia_diag = [mkdiag(wp, 128, fcol("ciac", t_, p=128)) for t_ in range(9)]
    cbp = tp.tile([128, 50 * BP], BF, name="cbp")
    pad_borders(cbp, 128, 50, BP, 9)
    cib_diag = [mkdiag(wp, 128, fcol("cibc", t_, p=128)) for t_ in range(49)]
    t_dense = tp.tile([C6, N], BF, name="t_dense")
    ci2v = tp.tile([C6, N], BF, name="ci2v")
    pprod = tp.tile([C6, N], BF, name="pprod")

    t_img = t_dense.rearrange("p (h w) -> p h w", w=W)
    for cchunk in range(NCH):
        ps = psD.tile([C6, CH], F32, name="dps", tag="dps")
        for k in range(3):
            nc.tensor.matmul(
                ps, lhsT=cpinT[k],
                rhs=at_sb[k][:, CH * cchunk:CH * (cchunk + 1)],
                start=(k == 0), stop=(k == 2))
        evac(t_dense[:, CH * cchunk:CH * (cchunk + 1)], ps, cp_in_b,
             accum=tsum[:, cchunk:cchunk + 1])
        # stream freshly-written t rows into the packed pad buffer
        r0, r1 = 8 * cchunk, 8 * cchunk + 8
        lo, hi = max(r0, 0), min(r1, 33)
        if lo < hi:
            sync.dma_start(
                out=_win(tpk, (lo + 1) * HP + 1, [[HP, hi - lo], [1, W]],
                         p=(0, 64)),
                in_=t_img[:, lo:hi, :])
        lo, hi = max(r0, 31), min(r1, 64)
        if lo < hi:
            sync.dma_start(
                out=_win(tpk, (lo - 31) * HP + 1, [[HP, hi - lo], [1, W]],
                         p=(64, 128)),
                in_=t_img[:, lo:hi, :])

    nc.vector.tensor_reduce(out=tm, in_=tsum, axis=AX.X, op=OP.add)
    nc.vector.tensor_scalar_mul(tm, tm, 1.0 / N)
    psc = psD.tile([C6, 1], F32, name="dps1", tag="dps")
    nc.tensor.matmul(psc, lhsT=ci1T, rhs=tm, start=True, stop=True)
    nc.vector.tensor_scalar(out=ci1v, in0=psc, scalar1=ci1_b,
                            scalar2=None, op0=OP.add)

    # ci2a (9 taps on PE) -> packed pad-9 buffer [128, 50*82]
    for cchunk in range(4):
        ps = psD.tile([128, CH], F32, name="dpsa", tag="dps")
        for t_ in range(9):
            dy, dx = divmod(t_, 3)
            src = _win(tpk, (8 * cchunk + dy) * HP + dx,
                       [[HP, 8], [1, W]])
            nc.tensor.matmul(ps, lhsT=cia_diag[t_], rhs=src,
                             start=(t_ == 0), stop=(t_ == 8))
        dst = _win(cbp, (8 * cchunk + 9) * BP + 9, [[BP, 8], [1, W]])
        evac(dst, ps, ci2a_b2)
    # halo exchange between halves
    sync.dma_start(
        out=_win(cbp, 41 * BP + 9, [[BP, 9], [1, W]], p=(0, 64)),
        in_=_win(cbp, 9 * BP + 9, [[BP, 9], [1, W]], p=(64, 128)))
    sync.dma_start(
        out=_win(cbp, 9, [[BP, 9], [1, W]], p=(64, 128)),
        in_=_win(cbp, 32 * BP + 9, [[BP, 9], [1, W]], p=(0, 64)))

    # ci2b 49 dilated taps on PE -> ci2c per chunk -> ci2v dense [64, N]
    for cchunk in range(4):
        ps = psD.tile([128, CH], F32, name="dpsb", tag="dps")
        for t_ in range(49):
            ty, tx = divmod(t_, 7)
            src = _win(cbp, (8 * cchunk + 3 * ty) * BP + 3 * tx,
                       [[BP, 8], [1, W]])
            nc.tensor.matmul(ps, lhsT=cib_diag[t_], rhs=src,
                             start=(t_ == 0), stop=(t_ == 48))
        cbd = cbd_ring[cchunk % 2]
        evac(cbd, ps, ci2b_b2)
        for half in range(2):
            ps2 = psD.tile([C6, CH], F32, name="dpsc", tag="dps")
            nc.tensor.matmul(ps2, lhsT=ci2cT[half], rhs=cbd,
                             start=True, stop=True)
            evac(ci2v[:, 2048 * half + CH * cchunk:
                      2048 * half + CH * (cchunk + 1)], ps2, ci2c_b)

    # pprod = t * ci1 * ci2 (chunked, accumulate mean for cm gate)
    for cchunk in range(NCH):
        sl = slice(CH * cchunk, CH * (cchunk + 1))
        nc.vector.scalar_tensor_tensor(
            out=pprod[:, sl], in0=t_dense[:, sl], scalar=ci1v,
            in1=ci2v[:, sl], op0=OP.mult, op1=OP.mult,
            accum_out=pm[:, cchunk:cchunk + 1])

    # cp_out fused into at; sp_out gate per chunk
    for cchunk in range(NCH):
        sl = slice(CH * cchunk, CH * (cchunk + 1))
        for m in range(3):
            ps = psD.tile([128, CH], F32, name="dpso", tag="dpso", bufs=2)
            nc.tensor.matmul(
                ps, lhsT=cpoutT[:, 128 * m:128 * (m + 1)],
                rhs=pprod[:, sl], start=True, stop=True)
            nc.vector.scalar_tensor_tensor(
                out=at_sb[m][:, sl], in0=ps, scalar=cp_out_b[m],
                in1=at_sb[m][:, sl], op0=OP.add, op1=OP.add)
        for m in range(3):
            ps = psE.tile([128, CH], F32, name="eps3", tag="eps")
            nc.tensor.matmul(
                ps, lhsT=spoutT[:, 128 * m:128 * (m + 1)],
                rhs=gg[:, sl], start=True, stop=True)
            sg = sg_ring[(3 * cchunk + m) % 3]
            nc.scalar.activation(out=sg, in_=ps, func=AF.Sigmoid,
                                 bias=sp_out_b[m], scale=1.0)
            nc.vector.tensor_mul(at_sb[m][:, sl], at_sb[m][:, sl], sg)

    # cm gate from mean(pprod) via linearity of cp_out
    nc.vector.tensor_reduce(out=pmean32, in_=pm, axis=AX.X, op=OP.add)
    nc.vector.tensor_scalar_mul(pmean, pmean32, 1.0 / N)
    for m in range(3):
        ps = psD.tile([128, 1], F32, name="dpsm", tag="dpso", bufs=2)
        nc.tensor.matmul(ps, lhsT=cpoutT[:, 128 * m:128 * (m + 1)],
                         rhs=pmean, start=True, stop=True)
        nc.scalar.activation(out=cm_sig[:, m:m + 1], in_=ps, func=AF.Sigmoid,
                             bias=cp_out_b[m], scale=1.0)
    cpstack.close()
    psEstack.close()

    # ==== final gates + proj + store =======================================
    with tc.tile_pool(name="psF", bufs=6, space="PSUM") as psF:
        for cchunk in range(NCH):
            sl = slice(CH * cchunk, CH * (cchunk + 1))
            for m in range(3):
                nc.vector.scalar_tensor_tensor(
                    out=at_sb[m][:, sl], in0=convx[m][:, sl],
                    scalar=cm_sig[:, m:m + 1], in1=at_sb[m][:, sl],
                    op0=OP.mult, op1=OP.add)
            for m in range(3):
                ps = psF.tile([128, CH], F32, name="fps", tag="fps")
                for k in range(3):
                    nc.tensor.matmul(
                        ps, lhsT=projT[k][:, 128 * m:128 * (m + 1)],
                        rhs=at_sb[k][:, sl],
                        start=(k == 0), stop=(k == 2))
                ot = oring[(3 * cchunk + m) % 4]
                evac(ot, ps, proj_b[m])
                sync.dma_start(
                    out=d["out"][128 * m:128 * (m + 1), sl], in_=ot)


def build_program():
    nc = bacc.Bacc("TRN2", target_bir_lowering=False, debug=False,
                   num_devices=NCORES)
    d = {}
    d["wt"] = nc.dram_tensor("wt", [3, 128, 3 * C], BF, kind="ExternalInput")
    d["blob16"] = nc.dram_tensor("blob16", [128, BCOLS], BF,
                                 kind="ExternalInput")
    d["blob32"] = nc.dram_tensor("blob32", [128, FCOLS], F32,
                                 kind="ExternalInput")
    d["x"] = nc.dram_tensor("x", [C, N], BF, kind="ExternalInput")
    d["out"] = nc.dram_tensor("out", [C, N], F32, kind="ExternalOutput")

    with tile.TileContext(nc) as tc:
        emit(tc, d)
    nc.compile()
    return nc


_cached = None


def kernel(**inputs) -> np.ndarray:
    global _last_results, _cached
    x = np.asarray(inputs["x"], np.float32)
    B = x.shape[0]
    assert x.shape == (NCORES, C, H, W), x.shape
    g = build_host_inputs(inputs)
    if _cached is None:
        _cached = build_program()
    nc = _cached

    base = {nm: np.ascontiguousarray(arr) for nm, arr in g.items()}
    in_maps = []
    for b in range(B):
        m = dict(base)
        m["x"] = np.ascontiguousarray(x[b].reshape(C, N)).astype(BF16)
        in_maps.append(m)

    trace = os.environ.get("KERNEL_TRACE") == "1"
    try:
        res = run_bass_kernel_spmd(nc, in_maps, list(range(NCORES)),
                                   trace=trace)
    except ModuleNotFoundError:
        res = run_bass_kernel_spmd(nc, in_maps, list(range(NCORES)),
                                   trace=False)
    _last_results = res
    out = np.stack([res.results[b]["out"].reshape(C, H, W) for b in range(B)])
    return out.astype(np.float32)
